# revision 47
# baseline (speedup 1.0000x reference)
"""Multi-head attention (B=2, S=2048, D=1024, H=16, HD=64) on 8 TRN2 NeuronCores.

Sharding: core c -> (batch b = c//4, head-group g = c%4 of 4 heads).
Each core computes its head-group's Q/K/V projections (Wq/Wk/Wv output-dim
slice), full attention over its 4 heads, and a partial output projection
through the matching Wo row slice.  The host sums the 4 head-group partials
per batch (the "all-reduce") and applies the exact bv/bo correction.

Device kernel layout (per core):
  X^T arrives pre-transposed (host marshals the shard) -> Q^T,K^T in [hd, s]
  (2 heads packed per 128-partition chunk), V in [s, hd] with a ones column
  per (head, s-chunk) so the PV matmul yields softmax denominators for free.
  scores^T = K^T-stationary matmul in [k, q] layout; exp on ACT straight out
  of PSUM (no max-subtract: inputs are unit-scale, scores ~N(0, 0.4^2));
  mask applied exactly by scaling V+ones rows with mask (zeroes masked keys
  out of both numerator and denominator).  ctx^T normalized by 1/den after
  PV; Y = ctx^T.T @ Wo.  Projections/scores/Y run in float32r (full PE rate
  at N>=256, 11-bit mantissa, fp32 PSUM accumulation); the S x S exp(P) and
  V tiles are bf16 so three q-block units fit in SBUF concurrently.
  Measured on HW: rel err 1.47e-3; ~300-330 us/core (loop-differenced),
  cost-model estimate 215 us.

  Emission is software-pipelined: the first three attention units (which
  need only the head-pair-0 Q^T/K^T chunks) are interleaved into the
  projection phase so ACT's exp stream starts ~50us earlier; each unit's
  PV matmuls interleave with the next unit's score matmuls, and each
  q-block's output-projection tiles issue as soon as its four heads'
  ctx^T chunks complete.  reps>1 wraps the body in a hardware loop (used
  only by the timing harness).
"""

import numpy as np

B, S, D = 2, 2048, 1024
H, HD = 16, 64
NCORES = 8
HG = 4             # heads per core
HDG = HG * HD      # 256 head-dims per core
P = 128
KC = S // P        # 16 key chunks
DC = D // P        # 8 contraction chunks for projections
NQ = 512           # q-block size
NJ = S // NQ       # 4 q-blocks
SCK = 2            # score psum tile spans 2 key-chunks -> [128, 1024]
NCG = KC // SCK    # 8 chunk-groups per unit
VW = HD + 1        # 65: V columns + ones column

# units pre-emitted during the projection phase (all use hc0 Q^T/K^T)
PRE_UNITS = ((0, 0), (1, 0), (2, 0), (3, 0))


def _build_program(reps=1):
    import concourse.bass as bass
    import concourse.mybir as mybir
    import concourse.tile as tile
    from concourse import bacc

    fp32 = mybir.dt.float32
    fp32r = mybir.dt.float32r
    bf16 = mybir.dt.bfloat16
    Act = mybir.ActivationFunctionType

    nc = bacc.Bacc("TRN2", target_bir_lowering=False, debug=False,
                   num_devices=NCORES)

    XT = nc.dram_tensor("XT", [D, S], fp32r, kind="ExternalInput").ap()
    WQ = nc.dram_tensor("WQ", [D, HDG], fp32r, kind="ExternalInput").ap()
    WK = nc.dram_tensor("WK", [D, HDG], fp32r, kind="ExternalInput").ap()
    WV = nc.dram_tensor("WV", [D, HDG], fp32r, kind="ExternalInput").ap()
    WO = nc.dram_tensor("WO", [HDG, D], fp32r, kind="ExternalInput").ap()
    BQ = nc.dram_tensor("BQ", [P, 2], fp32, kind="ExternalInput").ap()
    BK = nc.dram_tensor("BK", [P, 2], fp32, kind="ExternalInput").ap()
    MASKT = nc.dram_tensor("MASKT", [P, KC], fp32, kind="ExternalInput").ap()
    Y = nc.dram_tensor("Y", [S, D], fp32, kind="ExternalOutput").ap()

    from contextlib import ExitStack
    with tile.TileContext(nc) as tc, ExitStack() as _loop_stk, \
         tc.tile_pool(name="persist", bufs=1) as persist, \
         tc.tile_pool(name="expt_pool", bufs=3) as exptp, \
         tc.tile_pool(name="small", bufs=2) as small, \
         tc.tile_pool(name="yout", bufs=3) as yout, \
         tc.tile_pool(name="ctxp", bufs=2) as ctxp, \
         tc.tile_pool(name="scps", bufs=2, space="PSUM") as scps, \
         tc.tile_pool(name="pvps", bufs=2, space="PSUM") as pvps:

        if reps > 1:
            _loop_stk.enter_context(tc.For_i(0, reps, 1))
        bq_sb = persist.tile([P, 2], fp32)
        nc.sync.dma_start(bq_sb[:], BQ[:])
        bk_sb = persist.tile([P, 2], fp32)
        nc.sync.dma_start(bk_sb[:], BK[:])
        mask_sb = persist.tile([P, KC], fp32)
        nc.sync.dma_start(mask_sb[:], MASKT[:])

        # preload the exp table while input DMAs stream
        warm = persist.tile([1, 1], fp32)
        nc.scalar.activation(warm[:], bq_sb[0:1, 0:1], Act.Exp)

        qt_sb = [persist.tile([P, S], fp32r, name=f"qt{i}") for i in range(2)]
        kt_sb = [persist.tile([P, S], fp32r, name=f"kt{i}") for i in range(2)]
        vaug = persist.tile([P, HG * KC * VW], bf16)
        wo_sb = persist.tile([P, 2 * D], fp32r)

        expts = {}
        pvs = {}
        ctxts = {}

        def emit_scores_cg(h, jb, cg):
            hc, hp = h // 2, (h % 2) * 64
            if (h, jb) not in expts:
                expts[(h, jb)] = exptp.tile([P, KC * NQ], bf16, tag="expt",
                                            name=f"expt_{h}_{jb}")
            expt = expts[(h, jb)]
            sc = scps.tile([P, SCK * NQ], fp32, tag="sc",
                           name=f"sc_{h}_{jb}_{cg}")
            for u in range(SCK):
                c = cg * SCK + u
                nc.tensor.matmul(
                    sc[:, u * NQ:(u + 1) * NQ],
                    kt_sb[hc][hp:hp + 64, c * P:(c + 1) * P],
                    qt_sb[hc][hp:hp + 64, jb * NQ:(jb + 1) * NQ],
                    start=True, stop=True)
            nc.scalar.activation(
                expt[:, cg * SCK * NQ:(cg + 1) * SCK * NQ], sc[:], Act.Exp)

        def pv_begin(h, jb):
            pvs[(h, jb)] = pvps.tile([VW, NQ], fp32, tag="pv",
                                     name=f"pv_{h}_{jb}")

        def pv_cg(h, jb, cg):
            pv = pvs[(h, jb)]
            expt = expts[(h, jb)]
            for u in range(SCK):
                c = cg * SCK + u
                nc.tensor.matmul(
                    pv[:],
                    vaug[:, (h * KC + c) * VW:(h * KC + c + 1) * VW],
                    expt[:, c * NQ:(c + 1) * NQ],
                    start=(c == 0), stop=(c == KC - 1))

        def pv_end(h, jb):
            hc, hp = h // 2, (h % 2) * 64
            expts.pop((h, jb))
            pv = pvs.pop((h, jb))
            if (hc, jb) not in ctxts:
                ctxts[(hc, jb)] = ctxp.tile([P, NQ], fp32r, tag=f"ct{hc}",
                                            name=f"ctxt_{hc}_{jb}")
            ct = ctxts[(hc, jb)]
            r64 = small.tile([VW, NQ], fp32, tag="r64", bufs=1)
            nc.vector.reciprocal(r64[64:65, :], pv[64:65, :])
            r0 = small.tile([1, NQ], fp32, tag="r0", bufs=1)
            nc.sync.dma_start(r0[:], r64[64:65, :])
            rb = small.tile([64, NQ], fp32, tag="rb", bufs=1)
            nc.gpsimd.partition_broadcast(rb[:], r0[:])
            if hp == 0:
                nc.vector.tensor_mul(ct[0:64, :], pv[0:64, :], rb[:])
            else:
                stg = small.tile([64, NQ], fp32r, tag="stg", bufs=1)
                nc.vector.tensor_mul(stg[:], pv[0:64, :], rb[:])
                nc.sync.dma_start(ct[64:128, :], stg[:])

        def pv_unit(h, jb):
            pv_begin(h, jb)
            for cg in range(NCG):
                pv_cg(h, jb, cg)
            pv_end(h, jb)

        # ---- projections interleaved with early attention units ----
        with tc.tile_pool(name="xtp", bufs=1) as xtp, \
             tc.tile_pool(name="wld", bufs=1) as wld, \
             tc.tile_pool(name="projps", bufs=2, space="PSUM") as projps:

            wq_sb = wld.tile([P, DC * HDG], fp32r)
            wk_sb = wld.tile([P, DC * HDG], fp32r)
            wv_sb = wld.tile([P, DC * HDG], fp32r)
            xt = [xtp.tile([P, S], fp32r, name=f"xt{c}") for c in range(DC)]

            def load_w(w_sb, W, c):
                nc.sync.dma_start(w_sb[:, c * HDG:(c + 1) * HDG],
                                  W[c * P:(c + 1) * P, :])

            def load_xt(c, jb):
                nc.sync.dma_start(
                    xt[c][:, jb * NQ:(jb + 1) * NQ],
                    XT[c * P:(c + 1) * P, jb * NQ:(jb + 1) * NQ])

            # DMA order = first-use order; wq/xt-jb0 pairs so the first
            # projection group starts after ~2 transfers.
            for c in range(DC):
                load_w(wq_sb, WQ, c)
                load_xt(c, 0)
            for c in range(DC):
                load_w(wk_sb, WK, c)
            for c in range(DC):
                load_w(wv_sb, WV, c)
            for c in range(DC):
                load_xt(c, 1)
            for jb in range(2, NJ):
                for c in range(DC):
                    load_xt(c, jb)
            for c in range(2):
                nc.sync.dma_start(wo_sb[:, c * D:(c + 1) * D],
                                  WO[c * P:(c + 1) * P, :])

            def proj_group(w_sb, t_sb, b_sb, hc, jb, tag):
                pp = projps.tile([P, NQ], fp32, tag="pp",
                                 name=f"pp_{tag}_{hc}_{jb}")
                for c in range(DC):
                    nc.tensor.matmul(
                        pp[:],
                        w_sb[:, c * HDG + hc * P:c * HDG + (hc + 1) * P],
                        xt[c][:, jb * NQ:(jb + 1) * NQ],
                        start=(c == 0), stop=(c == DC - 1))
                nc.vector.tensor_scalar(
                    t_sb[hc][:, jb * NQ:(jb + 1) * NQ], pp[:],
                    b_sb[:, hc:hc + 1], None, mybir.AluOpType.add)

            def v_group(i):
                vp = projps.tile([P, NQ], fp32, tag="pp", name=f"vp_{i}")
                for c in range(DC):
                    nc.tensor.matmul(
                        vp[:, 0:HDG], xt[c][:, i * P:(i + 1) * P],
                        wv_sb[:, c * HDG:(c + 1) * HDG],
                        start=(c == 0), stop=(c == DC - 1))
                for h in range(HG):
                    oc = (h * KC + i) * VW + HD
                    nc.vector.tensor_copy(vaug[:, oc:oc + 1],
                                          mask_sb[:, i:i + 1])
                for h in range(HG):
                    vc = (h * KC + i) * VW
                    nc.vector.tensor_scalar(
                        vaug[:, vc:vc + HD], vp[:, h * HD:(h + 1) * HD],
                        mask_sb[:, i:i + 1], None, mybir.AluOpType.mult)

            # Arrival-ordered: per q-block K^T + first unit's scores;
            # then remaining Q^T/K^T with the other pre-units woven in;
            # V groups last (wv arrives late, nothing stalls on them
            # until the first PV).
            proj_group(wq_sb, qt_sb, bq_sb, 0, 0, "q")
            for jb in range(NJ):
                proj_group(wk_sb, kt_sb, bk_sb, 0, jb, "k")
                emit_scores_cg(0, 0, 2 * jb)
                emit_scores_cg(0, 0, 2 * jb + 1)
                for i in range(4 * jb, 4 * jb + 4):
                    v_group(i)
            for cg in range(0, NCG, 2):
                emit_scores_cg(1, 0, cg)
                emit_scores_cg(1, 0, cg + 1)
                proj_group(wq_sb, qt_sb, bq_sb, 1 if cg == 0 else 0,
                           [0, 1, 2, 3][cg // 2], "q")
            for jb in range(NJ):
                proj_group(wk_sb, kt_sb, bk_sb, 1, jb, "k")
                emit_scores_cg(2, 0, 2 * jb)
                emit_scores_cg(2, 0, 2 * jb + 1)
            pv_unit(0, 0)
            sc30 = [(0, 1), (2, 3), (4, 5, 6, 7)]
            for jb in range(1, NJ):
                proj_group(wq_sb, qt_sb, bq_sb, 1, jb, "q")
                for cg in sc30[jb - 1]:
                    emit_scores_cg(3, 0, cg)

        # ---- attention units + fused output projection ----
        with tc.tile_pool(name="yps", bufs=2, space="PSUM") as yps:

            def emit_y(jb):
                c0, c1 = ctxts.pop((0, jb)), ctxts.pop((1, jb))
                for m in range(jb * NQ // P, (jb + 1) * NQ // P):
                    mo = (m - jb * NQ // P) * P
                    for dh in range(2):
                        yp = yps.tile([P, 512], fp32, tag="yp",
                                      name=f"yp_{m}_{dh}")
                        for hc, ct in ((0, c0), (1, c1)):
                            nc.tensor.matmul(
                                yp[:],
                                ct[:, mo:mo + P],
                                wo_sb[:, hc * D + dh * 512:
                                      hc * D + (dh + 1) * 512],
                                start=(hc == 0), stop=(hc == 1))
                        ysb = yout.tile([P, 512], fp32, tag="ysb",
                                        name=f"ysb_{m}_{dh}")
                        nc.vector.tensor_copy(ysb[:], yp[:])
                        nc.sync.dma_start(
                            Y[m * P:(m + 1) * P, dh * 512:(dh + 1) * 512],
                            ysb[:])

            units = [(h, jb) for jb in range(NJ) for h in (2, 1, 3, 0)]
            pending_y = None
            for idx, (h, jb) in enumerate(units):
                nxt = units[idx + 1] if idx + 1 < len(units) else None
                if pending_y is not None:
                    emit_y(pending_y)
                    pending_y = None
                pv_begin(h, jb)
                for cg in range(NCG):
                    if nxt is not None and nxt not in PRE_UNITS:
                        emit_scores_cg(nxt[0], nxt[1], cg)
                    pv_cg(h, jb, cg)
                pv_end(h, jb)
                if idx % HG == HG - 1:
                    pending_y = jb
            emit_y(pending_y)

    nc.finalize()
    return nc


_program_cache = {}


def _get_program():
    if "nc" not in _program_cache:
        _program_cache["nc"] = _build_program()
    return _program_cache["nc"]


def _to_bf16(a):
    import ml_dtypes
    return np.ascontiguousarray(a, np.float32).astype(ml_dtypes.bfloat16)


def _round_fp32r(a):
    """Round fp32 -> fp32r (11-bit mantissa; low 12 bits zero), RNE."""
    b = np.ascontiguousarray(a, np.float32).view(np.uint32).copy()
    b += np.uint32(0x7FF) + ((b >> np.uint32(12)) & np.uint32(1))
    b &= np.uint32(0xFFFFF000)
    return b.view(np.float32)


def _make_in_maps(inputs):
    X = np.asarray(inputs["X"], np.float32)
    mask = np.asarray(inputs["mask"], np.float32)
    Wq = np.asarray(inputs["Wq"], np.float32)
    Wk = np.asarray(inputs["Wk"], np.float32)
    Wv = np.asarray(inputs["Wv"], np.float32)
    Wo = np.asarray(inputs["Wo"], np.float32)
    bq = np.asarray(inputs["bq"], np.float32)
    bk = np.asarray(inputs["bk"], np.float32)

    scale = np.float32(1.0 / np.sqrt(HD))
    in_maps = []
    for c in range(NCORES):
        b, g = c // HG, c % HG
        sl = slice(g * HDG, (g + 1) * HDG)
        in_maps.append({
            "XT": _round_fp32r(X[b].T),
            "WQ": _round_fp32r(Wq[:, sl] * scale),
            "WK": _round_fp32r(Wk[:, sl]),
            "WV": _round_fp32r(Wv[:, sl]),
            "WO": _round_fp32r(Wo[sl, :]),
            "BQ": np.ascontiguousarray((bq[sl] * scale).reshape(2, P).T),
            "BK": np.ascontiguousarray(bk[sl].reshape(2, P).T),
            "MASKT": np.ascontiguousarray(mask[b].reshape(KC, P).T),
        })
    return in_maps


def _run(inputs, trace=False, tmpdir=None):
    from concourse import bass_utils

    nc = _get_program()
    in_maps = _make_in_maps(inputs)
    res = bass_utils.run_bass_kernel_spmd(
        nc, in_maps, core_ids=list(range(NCORES)), trace=trace, tmpdir=tmpdir)

    bv = np.asarray(inputs["bv"], np.float32)
    bo = np.asarray(inputs["bo"], np.float32)
    Wo = np.asarray(inputs["Wo"], np.float32)
    row = bv @ Wo + bo  # exact bv/bo contribution (attn rows sum to 1)

    out = np.zeros((B, S, D), np.float32)
    for c in range(NCORES):
        out[c // HG] += res.results[c]["Y"]
    out += row[None, None, :]
    return out, res


def kernel(**inputs):
    out, _ = _run(inputs, trace=False)
    return out


# revision 52
# speedup vs baseline: 1.6151x; 1.6151x over previous
"""Multi-head attention (B=2, S=2048, D=1024, H=16, HD=64) on 8 TRN2 NeuronCores.

Sharding: core c -> (batch b = c//4, head-group g = c%4 of 4 heads).
Each core computes its head-group's Q/K/V projections (Wq/Wk/Wv output-dim
slice), full attention over its 4 heads, and a partial output projection
through the matching Wo row slice.  The host sums the 4 head-group partials
per batch (the "all-reduce") and applies the exact bv/bo correction.

Device kernel layout (per core):
  X^T arrives pre-transposed (host marshals the shard) -> Q^T,K^T in [hd, s]
  (2 heads packed per 128-partition chunk), V in [s, hd] with a ones column
  per (head, s-chunk) so the PV matmul yields softmax denominators for free.
  scores^T = K^T-stationary matmul in [k, q] layout; exp on ACT straight out
  of PSUM (no max-subtract: inputs are unit-scale, scores ~N(0, 0.4^2));
  mask applied exactly by scaling V+ones rows with mask (zeroes masked keys
  out of both numerator and denominator).  ctx^T normalized by 1/den after
  PV; Y = ctx^T.T @ Wo.  Projections/scores/Y run in float32r (full PE rate
  at N>=256, 11-bit mantissa, fp32 PSUM accumulation); the S x S exp(P) and
  V tiles are bf16 so three q-block units fit in SBUF concurrently.
  Measured on HW: rel err 1.47e-3; ~300-350 us/core (loop-differenced,
  incl. ~6us/rep loop overhead), cost-model estimate 209 us.

  Emission is software-pipelined: the first three attention units (which
  need only the head-pair-0 Q^T/K^T chunks) are interleaved into the
  projection phase so ACT's exp stream starts ~50us earlier; each unit's
  PV matmuls interleave with the next unit's score matmuls, and each
  q-block's output-projection tiles issue as soon as its four heads'
  ctx^T chunks complete.  reps>1 wraps the body in a hardware loop (used
  only by the timing harness).
"""

import numpy as np

B, S, D = 2, 2048, 1024
H, HD = 16, 64
NCORES = 8
HG = 4             # heads per core
HDG = HG * HD      # 256 head-dims per core
P = 128
KC = S // P        # 16 key chunks
DC = D // P        # 8 contraction chunks for projections
NQ = 512           # q-block size
NJ = S // NQ       # 4 q-blocks
SCK = 2            # score psum tile spans 2 key-chunks -> [128, 1024]
NCG = KC // SCK    # 8 chunk-groups per unit
VW = HD + 1        # 65: V columns + ones column

# units pre-emitted during the projection phase (all use hc0 Q^T/K^T)
PRE_UNITS = ((0, 0), (1, 0), (2, 0), (3, 0))


def _build_program(reps=1):
    import concourse.bass as bass
    import concourse.mybir as mybir
    import concourse.tile as tile
    from concourse import bacc

    fp32 = mybir.dt.float32
    fp32r = mybir.dt.float32r
    bf16 = mybir.dt.bfloat16
    Act = mybir.ActivationFunctionType

    nc = bacc.Bacc("TRN2", target_bir_lowering=False, debug=False,
                   num_devices=NCORES)

    XT = nc.dram_tensor("XT", [D, S], fp32r, kind="ExternalInput").ap()
    WQ = nc.dram_tensor("WQ", [D, HDG], fp32r, kind="ExternalInput").ap()
    WK = nc.dram_tensor("WK", [D, HDG], fp32r, kind="ExternalInput").ap()
    WV = nc.dram_tensor("WV", [D, HDG], fp32r, kind="ExternalInput").ap()
    WO = nc.dram_tensor("WO", [HDG, D], fp32r, kind="ExternalInput").ap()
    BQ = nc.dram_tensor("BQ", [P, 2], fp32, kind="ExternalInput").ap()
    BK = nc.dram_tensor("BK", [P, 2], fp32, kind="ExternalInput").ap()
    MASKT = nc.dram_tensor("MASKT", [P, KC], fp32, kind="ExternalInput").ap()
    Y = nc.dram_tensor("Y", [S, D], fp32, kind="ExternalOutput").ap()

    from contextlib import ExitStack
    with tile.TileContext(nc) as tc, ExitStack() as _loop_stk, \
         tc.tile_pool(name="persist", bufs=1) as persist, \
         tc.tile_pool(name="expt_pool", bufs=3) as exptp, \
         tc.tile_pool(name="small", bufs=2) as small, \
         tc.tile_pool(name="yout", bufs=3) as yout, \
         tc.tile_pool(name="ctxp", bufs=2) as ctxp, \
         tc.tile_pool(name="scps", bufs=2, space="PSUM") as scps, \
         tc.tile_pool(name="pvps", bufs=2, space="PSUM") as pvps:

        if reps > 1:
            _loop_stk.enter_context(tc.For_i(0, reps, 1))
        bq_sb = persist.tile([P, 2], fp32)
        bk_sb = persist.tile([P, 2], fp32)
        mask_sb = persist.tile([P, KC], fp32)

        qt_sb = [persist.tile([P, S], fp32r, name=f"qt{i}") for i in range(2)]
        kt_sb = [persist.tile([P, S], fp32r, name=f"kt{i}") for i in range(2)]
        vaug = persist.tile([P, HG * KC * VW], bf16)
        wo_sb = persist.tile([P, 2 * D], fp32r)

        expts = {}
        pvs = {}
        ctxts = {}

        def emit_scores_cg(h, jb, cg):
            hc, hp = h // 2, (h % 2) * 64
            if (h, jb) not in expts:
                expts[(h, jb)] = exptp.tile([P, KC * NQ], bf16, tag="expt",
                                            name=f"expt_{h}_{jb}")
            expt = expts[(h, jb)]
            sc = scps.tile([P, SCK * NQ], fp32, tag="sc",
                           name=f"sc_{h}_{jb}_{cg}")
            for u in range(SCK):
                c = cg * SCK + u
                nc.tensor.matmul(
                    sc[:, u * NQ:(u + 1) * NQ],
                    kt_sb[hc][hp:hp + 64, c * P:(c + 1) * P],
                    qt_sb[hc][hp:hp + 64, jb * NQ:(jb + 1) * NQ],
                    start=True, stop=True)
            nc.scalar.activation(
                expt[:, cg * SCK * NQ:(cg + 1) * SCK * NQ], sc[:], Act.Exp)

        def pv_begin(h, jb):
            pvs[(h, jb)] = pvps.tile([VW, NQ], fp32, tag="pv",
                                     name=f"pv_{h}_{jb}")

        def pv_cg(h, jb, cg):
            pv = pvs[(h, jb)]
            expt = expts[(h, jb)]
            for u in range(SCK):
                c = cg * SCK + u
                nc.tensor.matmul(
                    pv[:],
                    vaug[:, (h * KC + c) * VW:(h * KC + c + 1) * VW],
                    expt[:, c * NQ:(c + 1) * NQ],
                    start=(c == 0), stop=(c == KC - 1))

        def pv_end(h, jb):
            hc, hp = h // 2, (h % 2) * 64
            expts.pop((h, jb))
            pv = pvs.pop((h, jb))
            if (hc, jb) not in ctxts:
                ctxts[(hc, jb)] = ctxp.tile([P, NQ], fp32r, tag=f"ct{hc}",
                                            name=f"ctxt_{hc}_{jb}")
            ct = ctxts[(hc, jb)]
            r64 = small.tile([VW, NQ], fp32, tag="r64", bufs=1)
            nc.vector.reciprocal(r64[64:65, :], pv[64:65, :])
            # row-shift the recip into this tile's unused row 0
            nc.sync.dma_start(r64[0:1, :], r64[64:65, :])
            rb = small.tile([64, NQ], fp32, tag="rb", bufs=2)
            nc.gpsimd.partition_broadcast(rb[:], r64[0:1, :])
            if hp == 0:
                nc.vector.tensor_mul(ct[0:64, :], pv[0:64, :], rb[:])
            else:
                stg = small.tile([64, NQ], fp32r, tag="stg", bufs=1)
                nc.vector.tensor_mul(stg[:], pv[0:64, :], rb[:])
                nc.sync.dma_start(ct[64:128, :], stg[:])

        def pv_unit(h, jb):
            pv_begin(h, jb)
            for cg in range(NCG):
                pv_cg(h, jb, cg)
            pv_end(h, jb)

        # ---- projections interleaved with early attention units ----
        with tc.tile_pool(name="xtp", bufs=1) as xtp, \
             tc.tile_pool(name="wld", bufs=1) as wld, \
             tc.tile_pool(name="projps", bufs=2, space="PSUM") as projps:

            wq_sb = wld.tile([P, DC * HDG], fp32r)
            wk_sb = wld.tile([P, DC * HDG], fp32r)
            wv_sb = wld.tile([P, DC * HDG], fp32r)
            xt = [xtp.tile([P, S], fp32r, name=f"xt{c}") for c in range(DC)]

            def load_w(w_sb, W, c):
                nc.sync.dma_start(w_sb[:, c * HDG:(c + 1) * HDG],
                                  W[c * P:(c + 1) * P, :])

            def load_xt(c, jb):
                nc.sync.dma_start(
                    xt[c][:, jb * NQ:(jb + 1) * NQ],
                    XT[c * P:(c + 1) * P, jb * NQ:(jb + 1) * NQ])

            # DMA order = first-use order; wq/xt-jb0 pairs so the first
            # projection group starts after ~2 transfers.
            for c in range(DC):
                load_w(wq_sb, WQ, c)
                load_xt(c, 0)
            nc.sync.dma_start(bq_sb[:], BQ[:])
            nc.sync.dma_start(bk_sb[:], BK[:])
            nc.sync.dma_start(mask_sb[:], MASKT[:])
            # preload the exp table while input DMAs stream
            warm = persist.tile([1, 1], fp32)
            nc.scalar.activation(warm[:], bq_sb[0:1, 0:1], Act.Exp)
            for c in range(DC):
                load_w(wk_sb, WK, c)
            for c in range(DC):
                load_w(wv_sb, WV, c)
            for c in range(DC):
                load_xt(c, 1)
            for jb in range(2, NJ):
                for c in range(DC):
                    load_xt(c, jb)
            for c in range(2):
                nc.sync.dma_start(wo_sb[:, c * D:(c + 1) * D],
                                  WO[c * P:(c + 1) * P, :])

            def proj_group(w_sb, t_sb, b_sb, hc, jb, tag):
                pp = projps.tile([P, NQ], fp32, tag="pp",
                                 name=f"pp_{tag}_{hc}_{jb}")
                for c in range(DC):
                    nc.tensor.matmul(
                        pp[:],
                        w_sb[:, c * HDG + hc * P:c * HDG + (hc + 1) * P],
                        xt[c][:, jb * NQ:(jb + 1) * NQ],
                        start=(c == 0), stop=(c == DC - 1))
                nc.vector.tensor_scalar(
                    t_sb[hc][:, jb * NQ:(jb + 1) * NQ], pp[:],
                    b_sb[:, hc:hc + 1], None, mybir.AluOpType.add)

            def v_group(i):
                vp = projps.tile([P, NQ], fp32, tag="pp", name=f"vp_{i}")
                for c in range(DC):
                    nc.tensor.matmul(
                        vp[:, 0:HDG], xt[c][:, i * P:(i + 1) * P],
                        wv_sb[:, c * HDG:(c + 1) * HDG],
                        start=(c == 0), stop=(c == DC - 1))
                for h in range(HG):
                    oc = (h * KC + i) * VW + HD
                    nc.vector.tensor_copy(vaug[:, oc:oc + 1],
                                          mask_sb[:, i:i + 1])
                for h in range(HG):
                    vc = (h * KC + i) * VW
                    nc.vector.tensor_scalar(
                        vaug[:, vc:vc + HD], vp[:, h * HD:(h + 1) * HD],
                        mask_sb[:, i:i + 1], None, mybir.AluOpType.mult)

            # Arrival-ordered: per q-block K^T + first unit's scores;
            # then remaining Q^T/K^T with the other pre-units woven in;
            # V groups last (wv arrives late, nothing stalls on them
            # until the first PV).
            proj_group(wq_sb, qt_sb, bq_sb, 0, 0, "q")
            for jb in range(NJ):
                proj_group(wk_sb, kt_sb, bk_sb, 0, jb, "k")
                emit_scores_cg(0, 0, 2 * jb)
                emit_scores_cg(0, 0, 2 * jb + 1)
                for i in range(4 * jb, 4 * jb + 4):
                    v_group(i)
            for cg in range(0, NCG, 2):
                emit_scores_cg(1, 0, cg)
                emit_scores_cg(1, 0, cg + 1)
                proj_group(wq_sb, qt_sb, bq_sb, 1 if cg == 0 else 0,
                           [0, 1, 2, 3][cg // 2], "q")
            for jb in range(NJ):
                proj_group(wk_sb, kt_sb, bk_sb, 1, jb, "k")
                emit_scores_cg(2, 0, 2 * jb)
                emit_scores_cg(2, 0, 2 * jb + 1)
            pv_unit(0, 0)
            sc30 = [(0, 1), (2, 3), (4, 5, 6, 7)]
            for jb in range(1, NJ):
                proj_group(wq_sb, qt_sb, bq_sb, 1, jb, "q")
                for cg in sc30[jb - 1]:
                    emit_scores_cg(3, 0, cg)

        # ---- attention units + fused output projection ----
        with tc.tile_pool(name="yps", bufs=2, space="PSUM") as yps:

            def emit_y(jb):
                c0, c1 = ctxts.pop((0, jb)), ctxts.pop((1, jb))
                for m in range(jb * NQ // P, (jb + 1) * NQ // P):
                    mo = (m - jb * NQ // P) * P
                    for dh in range(2):
                        yp = yps.tile([P, 512], fp32, tag="yp",
                                      name=f"yp_{m}_{dh}")
                        for hc, ct in ((0, c0), (1, c1)):
                            nc.tensor.matmul(
                                yp[:],
                                ct[:, mo:mo + P],
                                wo_sb[:, hc * D + dh * 512:
                                      hc * D + (dh + 1) * 512],
                                start=(hc == 0), stop=(hc == 1))
                        ysb = yout.tile([P, 512], fp32, tag="ysb",
                                        name=f"ysb_{m}_{dh}")
                        nc.vector.tensor_copy(ysb[:], yp[:])
                        nc.sync.dma_start(
                            Y[m * P:(m + 1) * P, dh * 512:(dh + 1) * 512],
                            ysb[:])

            units = [(h, jb) for jb in range(NJ) for h in (2, 1, 3, 0)]
            pending_y = None
            for idx, (h, jb) in enumerate(units):
                nxt = units[idx + 1] if idx + 1 < len(units) else None
                if pending_y is not None:
                    emit_y(pending_y)
                    pending_y = None
                pv_begin(h, jb)
                for cg in range(NCG):
                    if nxt is not None and nxt not in PRE_UNITS:
                        emit_scores_cg(nxt[0], nxt[1], cg)
                    pv_cg(h, jb, cg)
                pv_end(h, jb)
                if idx % HG == HG - 1:
                    pending_y = jb
            emit_y(pending_y)

    nc.finalize()
    return nc


_program_cache = {}


def _get_program():
    if "nc" not in _program_cache:
        _program_cache["nc"] = _build_program()
    return _program_cache["nc"]


def _to_bf16(a):
    import ml_dtypes
    return np.ascontiguousarray(a, np.float32).astype(ml_dtypes.bfloat16)


def _round_fp32r(a):
    """Round fp32 -> fp32r (11-bit mantissa; low 12 bits zero), RNE."""
    b = np.ascontiguousarray(a, np.float32).view(np.uint32).copy()
    b += np.uint32(0x7FF) + ((b >> np.uint32(12)) & np.uint32(1))
    b &= np.uint32(0xFFFFF000)
    return b.view(np.float32)


def _make_in_maps(inputs):
    X = np.asarray(inputs["X"], np.float32)
    mask = np.asarray(inputs["mask"], np.float32)
    Wq = np.asarray(inputs["Wq"], np.float32)
    Wk = np.asarray(inputs["Wk"], np.float32)
    Wv = np.asarray(inputs["Wv"], np.float32)
    Wo = np.asarray(inputs["Wo"], np.float32)
    bq = np.asarray(inputs["bq"], np.float32)
    bk = np.asarray(inputs["bk"], np.float32)

    scale = np.float32(1.0 / np.sqrt(HD))
    in_maps = []
    for c in range(NCORES):
        b, g = c // HG, c % HG
        sl = slice(g * HDG, (g + 1) * HDG)
        in_maps.append({
            "XT": _round_fp32r(X[b].T),
            "WQ": _round_fp32r(Wq[:, sl] * scale),
            "WK": _round_fp32r(Wk[:, sl]),
            "WV": _round_fp32r(Wv[:, sl]),
            "WO": _round_fp32r(Wo[sl, :]),
            "BQ": np.ascontiguousarray((bq[sl] * scale).reshape(2, P).T),
            "BK": np.ascontiguousarray(bk[sl].reshape(2, P).T),
            "MASKT": np.ascontiguousarray(mask[b].reshape(KC, P).T),
        })
    return in_maps


def _run(inputs, trace=False, tmpdir=None):
    from concourse import bass_utils

    nc = _get_program()
    in_maps = _make_in_maps(inputs)
    res = bass_utils.run_bass_kernel_spmd(
        nc, in_maps, core_ids=list(range(NCORES)), trace=trace, tmpdir=tmpdir)

    bv = np.asarray(inputs["bv"], np.float32)
    bo = np.asarray(inputs["bo"], np.float32)
    Wo = np.asarray(inputs["Wo"], np.float32)
    row = bv @ Wo + bo  # exact bv/bo contribution (attn rows sum to 1)

    out = np.zeros((B, S, D), np.float32)
    for c in range(NCORES):
        out[c // HG] += res.results[c]["Y"]
    out += row[None, None, :]
    return out, res


def kernel(**inputs):
    out, _ = _run(inputs, trace=False)
    return out
